# revision 55
# baseline (speedup 1.0000x reference)
"""MiniMax MoE gate (sigmoid + bias, top-8, normalized weights) on 8 TRN2 cores.

Full inputs in, full outputs out. Data-parallel over tokens: each core gets
1024 of the 8192 tokens; gate weight and bias are replicated.

Final strategy (DMA-roofline oriented; HW-measured 61.6us vs 86.9us
baseline, rel err 8.4e-3 vs the 2e-2 gate):
  - x ships as fp16 hi (2B) + scaled-fp8 lo (1B) = 3 B/elem instead of the
    4 B/elem bf16 hi/lo pair: xl8 = e4m3((x - fp16(x)) * 2^13). The third
    matmul term runs mixed-dtype (fp8 stationary x bf16 moving W*2^-13), so
    the product lands at the exact fp32 scale and accumulates into the same
    PSUM tile - no epilogue fixup. fp16 hi (10 mantissa bits vs bf16's 7)
    keeps the gate error at baseline level while cutting x bytes 25%
    (14.2MB/core total, ~40us at the ~358GB/s per-core HBM limit).
  - Per (chunk c, tile i) the PE runs TWO matmuls instead of three:
      mm1: xh[c,i] (stationary) x [Wh[c] | Wl[c]] (moving, N=128) -> psum[:,0:128]
      mm2: xl8[c,i] (stationary) x Wh[c]*2^-13 (moving bf16, N=64) -> psum[:,0:64]
    (one LDWEIGHTS covers both W terms). Epilogue folds the halves:
    logits = psum[:,0:64] + psum[:,64:128] (ACT copy stages one half to
    SBUF - DVE can read only one PSUM operand per op).
  - ALL input DMA rides the single SP HWDGE queue in exact PE-consumption
    order: W head, group-0 head, W tail, then per group interleaved hi/lo
    pieces. One queue sustains ~390GB/s; two queues race and starve the
    gating transfers, and SWDGE (gpsimd) hogs the SDMA engines. ACT
    carries only the outputs + epilogue ops; measured ACT_TABLE_LOAD and
    descriptor-gen (~0.6us per dma_start) otherwise delay the stream.
  - Per piece the PE runs all mm1s then all mm2s (the lo piece streams
    during the mm1 phase); the last piece of each group runs mm2s first
    so only its mm1s trail the final bytes. Each DMA piece gets its own
    SBUF tile. Small final groups shrink the post-DMA tail.
  - measured caveat: HWDGE dma_start triggers issue through 8 round-robin
    completion lanes, so only ~8 transfers can be in flight; piece sizes
    below ~0.5MB couple the stream to PE progress and slow it.
"""

import os

import numpy as np
import ml_dtypes

import concourse.bacc as bacc
import concourse.mybir as mybir
from concourse.bass_utils import run_bass_kernel_spmd
from concourse.tile import TileContext

T, D, E, K = 8192, 4096, 64, 8
NCORES = 8
P = 128
F32 = mybir.dt.float32
BF16 = mybir.dt.bfloat16
FP16 = mybir.dt.float16
FP8 = mybir.dt.float8e4
BF16_NP = ml_dtypes.bfloat16
FP8_NP = ml_dtypes.float8_e4m3fn
DC = D // P   # 32 contraction chunks
S_LO = 2.0 ** 13   # fp8 lo scale; w2 = Wh * 2^-13 exactly cancels it

# "fp8": x lo as scaled e4m3 (3 B/elem total); "bf16": x lo as bf16 (4 B/elem)
KMODE = os.environ.get("KMODE", "fp8")
# tiles (x128 tokens) per token-group, per core
GROUP_PLAN = tuple(int(v) for v in os.environ.get("KPLAN", "3,3,1,1").split(","))
# chunk-piece size of a group's x-hi stream: big groups get fine pieces
# (plenty of ring backlog), 1-tile groups coarser ones so the descriptor
# generator stays ahead of the drain
HI_SUB_BIG = int(os.environ.get("KSUBB", "8"))
HI_SUB_SMALL = int(os.environ.get("KSUBS", "16"))


def _plans(ts):
    """Per-group hi-piece chunk splits and lo-piece splits.

    Lean in the middle (fewest transfers - every transfer costs a trip
    through the 8-lane trigger window), fine at the head (PE start) and
    tail (post-DMA matmul latency).
    """
    nt = ts // P
    plan = list(GROUP_PLAN)
    if sum(plan) != nt:  # fallback for small test shards
        plan = [1] * nt
    subs, los = [], []
    for h, ntg in enumerate(plan):
        if h == 0:
            subs.append(tuple(HI_SUB_BIG for _ in range(DC // HI_SUB_BIG)))
            los.append(((0, DC // 2), (DC // 2, DC)))
        elif h == len(plan) - 1:
            subs.append((16, 8, 8))  # fine tail pieces
            los.append(((0, DC // 2), (DC // 2, DC)))
        else:
            subs.append((DC,))   # mid groups: one transfer per stream
            los.append(((0, DC),))
    return plan, subs, los


def build_nc(ts):
    """Per-core program for a shard of `ts` tokens."""
    plan, subs, los = _plans(ts)
    nh = len(plan)
    ths = [p * P for p in plan]
    total_cols = DC * sum(ths)
    lo_dt, lo_np_bytes = (FP8, 1) if KMODE == "fp8" else (BF16, 2)

    nc = bacc.Bacc("TRN2", target_bir_lowering=False)
    # host-tiled layout (see prepare_in_maps): group blocks side by side;
    # within a group, row p holds all of partition p's data contiguously.
    xhd = nc.dram_tensor("xt_hi", [P, total_cols], FP16, kind="ExternalInput")
    xld = nc.dram_tensor("xt_lo", [P, total_cols], lo_dt, kind="ExternalInput")
    # W arena in stream order: [whl c0-8 | w2 c0-8 | whl c8-32 | w2 c8-32]
    # so the head and tail each load with ONE contiguous transfer
    WHH, W2H = 8 * 2 * E, 8 * E
    WHT, W2T = (DC - 8) * 2 * E, (DC - 8) * E
    wad = nc.dram_tensor(
        "wt_all", [P, WHH + W2H + WHT + W2T], BF16, kind="ExternalInput"
    )
    b = nc.dram_tensor("bias", [1, E], F32, kind="ExternalInput")
    # idx (8 x int32) and weights (8 x fp32-bitcast) pack into one row so
    # each group stores with a single DMA; the host splits them
    opd = nc.dram_tensor("out_pk", [ts, 2 * K], mybir.dt.int32, kind="ExternalOutput")

    with TileContext(nc) as tc:
        with (
            tc.tile_pool(name="const", bufs=1) as cpool,
            tc.tile_pool(name="xin", bufs=1) as xpool,
            tc.tile_pool(name="epi", bufs=4) as epool,
            tc.tile_pool(name="outb", bufs=3) as opool,
            tc.tile_pool(name="plogit", bufs=2, space="PSUM") as plogit,
        ):
            bias_row = cpool.tile([1, E], F32)
            nc.gpsimd.dma_start(out=bias_row, in_=b[:, :])
            bias_bc = cpool.tile([P, E], F32)
            nc.gpsimd.partition_broadcast(bias_bc, bias_row)

            # EVERYTHING streams on the single SP HWDGE queue, in exact PE
            # consumption order - one queue sustains the per-core HBM rate,
            # and its FIFO guarantees bytes land in the order the PE needs
            # them (two queues race and starve the gating transfers; SWDGE
            # hogs the SDMA engines). ACT carries no transfers at all: its
            # sequencer runs only the epilogue copies/sigmoids.
            wa = cpool.tile([P, WHH + W2H + WHT + W2T], BF16)
            whl_h = wa[:, 0:WHH].rearrange("p (c e) -> p c e", e=2 * E)
            w2_h = wa[:, WHH:WHH + W2H].rearrange("p (c e) -> p c e", e=E)
            whl_t = wa[:, WHH + W2H:WHH + W2H + WHT].rearrange(
                "p (c e) -> p c e", e=2 * E
            )
            w2_t = wa[:, WHH + W2H + WHT:].rearrange("p (c e) -> p c e", e=E)

            def WHL(c):
                return whl_h[:, c, :] if c < 8 else whl_t[:, c - 8, :]

            def W2V(c):
                return w2_h[:, c, :] if c < 8 else w2_t[:, c - 8, :]
            # One SBUF tile PER DMA PIECE: a piece's transfer then has no
            # relation to reads of other pieces, so the DMA stream never
            # waits on PE progress (tile-granular hazard tracking would
            # otherwise couple piece N+1's write to piece N's reads).
            xhps, xlps, hi_rs = [], [], []
            offs = []
            off = 0
            for h in range(nh):
                th = ths[h]
                rs = []
                c0 = 0
                for cs in subs[h]:
                    rs.append((c0, c0 + cs))
                    c0 += cs
                hi_rs.append(rs)
                xhps.append([
                    xpool.tile(
                        [P, b - a, th], FP16, tag=f"xh{h}_{pi}",
                        name=f"xh{h}_{pi}",
                    )
                    for pi, (a, b) in enumerate(rs)
                ])
                xlps.append([
                    xpool.tile(
                        [P, b - a, th], lo_dt, tag=f"xl{h}_{qi}",
                        name=f"xl{h}_{qi}",
                    )
                    for qi, (a, b) in enumerate(los[h])
                ])
                offs.append(off)
                off += DC * th

            def ld_x(t, dram, h, c0, c1, eng):   # x piece tile of group h
                th = ths[h]
                eng.dma_start(
                    out=t[:].rearrange("p c t -> p (c t)"),
                    in_=dram[:, offs[h] + c0 * th: offs[h] + c1 * th],
                )

            # W head + group-0 head first so matmuls start ASAP, then W
            # tail, then the groups in PE order. Within a group the hi and
            # lo pieces interleave in consumption order: the PE runs "all
            # mm1s of a piece, then its mm2s", so each lo piece streams
            # while the mm1s of its range execute. The final hi piece of a
            # group lands after the final lo piece (its mm1s run last).
            def x_pieces(h):
                hi = hi_rs[h]
                order = []
                li = 0
                lo = los[h]
                for pi, (a, b) in enumerate(hi):
                    if pi == len(hi) - 1 and li < len(lo):
                        order.extend(("lo", qi) for qi in range(li, len(lo)))
                        li = len(lo)
                    order.append(("hi", pi))
                    while li < len(lo) and lo[li][0] < b:
                        order.append(("lo", li))
                        li += 1
                return order

            # All input on the single SP ring in consumption order. Every
            # multi-ring variant measured slower (bulk hi/lo split 67.8us,
            # SWDGE lo-stream 70us, strict per-piece alternation 80.5us,
            # vs 62.8us single-ring): the second ring contends for the
            # shared SDMA engines and breaks the consumption-order FIFO
            # guarantee that keeps the gating transfers first.
            def emit_group(h, skip=0):
                for kind, pi in x_pieces(h)[skip:]:
                    if kind == "hi":
                        a, bb = hi_rs[h][pi]
                        ld_x(xhps[h][pi], xhd, h, a, bb, nc.sync)
                    else:
                        a, bb = los[h][pi]
                        ld_x(xlps[h][pi], xld, h, a, bb, nc.sync)
                    yield

            # (A SWDGE head-prefetch during the startup window measured
            # 64.6us vs 62.6us - the Q7 path does not actually beat the
            # ring and contends with it. Keep everything on the ring.)
            nc.sync.dma_start(
                out=wa[:, 0:WHH + W2H], in_=wad[:, 0:WHH + W2H]
            )
            for n, _ in enumerate(emit_group(0)):
                if n == 1:
                    # W tail right after the first hi+lo pieces
                    nc.sync.dma_start(
                        out=wa[:, WHH + W2H:], in_=wad[:, WHH + W2H:]
                    )
            for h in range(1, nh):
                for _ in emit_group(h):
                    pass

            tok0 = 0
            for h in range(nh):
                th, ntg = ths[h], plan[h]
                lgs = [
                    plogit.tile([P, 2 * E], F32, tag=f"lg{i}", name=f"lg_h{h}_{i}")
                    for i in range(ntg)
                ]
                # two-phase per piece, in DMA-arrival order: all mm1s of a
                # piece run while the matching lo piece streams in, then the
                # mm2s. Last piece reversed (mm2s first), so the only work
                # after a group's final hi bytes land is that piece's mm1s,
                # and each tile's final write is its stop-flagged mm1.
                for pi, (a, b) in enumerate(hi_rs[h]):
                    last_piece = pi == len(hi_rs[h]) - 1

                    def mm(ph, c, i, lg):
                        tsl = slice(i * P, (i + 1) * P)
                        if ph == "hi":
                            nc.tensor.matmul(
                                lg[:, :],
                                xhps[h][pi][:, c - a, tsl],
                                WHL(c),
                                start=(c == 0),
                                stop=(c == DC - 1),
                            )
                        else:
                            qi = next(
                                q for q, (la, lb) in enumerate(los[h])
                                if la <= c < lb
                            )
                            nc.tensor.matmul(
                                lg[:, 0:E],
                                xlps[h][qi][:, c - los[h][qi][0], tsl],
                                W2V(c),
                                start=False, stop=False,
                            )

                    single = len(hi_rs[h]) == 1
                    if single:
                        # one hi transfer for the whole group: mm1(c0) first
                        # (it carries start=True and must precede any
                        # accumulate into the tile), then the mm2s, then the
                        # remaining mm1s tile-major (stop rides c31 last)
                        for i, lg in enumerate(lgs):
                            mm("hi", 0, i, lg)
                        for c in range(a, b):
                            for i, lg in enumerate(lgs):
                                mm("lo", c, i, lg)
                        for i, lg in enumerate(lgs):
                            for c in range(1, b):
                                mm("hi", c, i, lg)
                    elif not last_piece:
                        for ph in ("hi", "lo"):
                            for c in range(a, b):
                                for i, lg in enumerate(lgs):
                                    mm(ph, c, i, lg)
                    else:
                        # lo phase first (its bytes landed earlier), then the
                        # hi phase TILE-major so tile 0's stop-matmul retires
                        # first and its epilogue releases ASAP
                        for c in range(a, b):
                            for i, lg in enumerate(lgs):
                                mm("lo", c, i, lg)
                        for i, lg in enumerate(lgs):
                            for c in range(a, b):
                                mm("hi", c, i, lg)

                obuf = opool.tile(
                    [P, ntg, 2 * K], mybir.dt.uint32, tag=f"ob{ntg}", name=f"ob{h}"
                )
                for i, lg in enumerate(lgs):
                    # only one PSUM operand allowed per DVE op: stage the Wl
                    # half through SBUF on the (idle) ACT engine
                    wlh = epool.tile([P, E], F32, tag="wlh")
                    nc.scalar.copy(out=wlh, in_=lg[:, E:2 * E])
                    ls = epool.tile([P, E], F32, tag="ls")
                    nc.vector.tensor_tensor(
                        out=ls, in0=lg[:, 0:E], in1=wlh,
                        op=mybir.AluOpType.add,
                    )
                    sc = epool.tile([P, E], F32, tag="sc")
                    nc.scalar.activation(
                        out=sc, in_=ls,
                        func=mybir.ActivationFunctionType.Sigmoid,
                    )
                    bi = epool.tile([P, E], F32, tag="bi")
                    nc.vector.tensor_tensor(
                        out=bi, in0=sc, in1=bias_bc, op=mybir.AluOpType.add
                    )
                    msel = epool.tile([P, K], F32, tag="msel")
                    nc.vector.max(out=msel, in_=bi)
                    nc.vector.max_index(
                        out=obuf[:, i, 0:K], in_max=msel, in_values=bi
                    )
                    # one-hot extraction of the selected raw scores, batched
                    # over all 8 slots with 0-stride broadcast APs. The
                    # compare runs in fp32 (tie-exactness feeds idx), but
                    # the mask/score product runs in bf16 for 2x DVE rate -
                    # it only perturbs the WEIGHTS (~2^-9 rel), which have
                    # 10x headroom vs the idx-dominated gate.
                    bi_b = bi[:].rearrange("p (o e) -> p o e", o=1).to_broadcast(
                        [P, K, E]
                    )
                    sc16 = epool.tile([P, E], BF16, tag="sc16")
                    nc.scalar.copy(out=sc16, in_=sc)
                    sc_b = sc16[:].rearrange(
                        "p (o e) -> p o e", o=1
                    ).to_broadcast([P, K, E])
                    msel_b = msel[:].rearrange(
                        "p (k o) -> p k o", o=1
                    ).to_broadcast([P, K, E])
                    oh8 = epool.tile([P, K, E], BF16, tag="oh8")
                    nc.vector.tensor_tensor(
                        out=oh8, in0=bi_b, in1=msel_b,
                        op=mybir.AluOpType.is_equal,
                    )
                    ohs = epool.tile([P, K, E], BF16, tag="ohs")
                    nc.vector.tensor_tensor(
                        out=ohs, in0=oh8, in1=sc_b, op=mybir.AluOpType.mult
                    )
                    ssel = epool.tile([P, K], F32, tag="ssel")
                    nc.vector.tensor_reduce(
                        out=ssel, in_=ohs,
                        axis=mybir.AxisListType.X, op=mybir.AluOpType.add,
                    )
                    ssum = epool.tile([P, 1], F32, tag="ssum")
                    nc.vector.tensor_reduce(
                        out=ssum, in_=ssel,
                        axis=mybir.AxisListType.X, op=mybir.AluOpType.add,
                    )
                    rsum = epool.tile([P, 1], F32, tag="rsum")
                    nc.vector.reciprocal(out=rsum, in_=ssum)
                    nc.vector.tensor_scalar_mul(
                        obuf[:, i, K:2 * K].bitcast(F32), ssel, rsum[:]
                    )

                # token at output partition q of tile i is tok0 + q*ntg + i.
                # Outputs ride the ACT ring (empty, so they land promptly)
                # and keep the SP ring pure-input.
                nc.scalar.dma_start(
                    out=opd[tok0:tok0 + th, :].rearrange(
                        "(q i) k -> q i k", i=ntg
                    ),
                    in_=obuf[:].bitcast(mybir.dt.int32),
                )
                tok0 += th

    nc.compile()
    return nc


_NC_CACHE = {}


def _get_nc(ts):
    if ts not in _NC_CACHE:
        _NC_CACHE[ts] = build_nc(ts)
    return _NC_CACHE[ts]


def _tile_xt(xs, ts):
    """[ts, D] fp32 -> [P, DC*ts] fp32 in the device layout.

    Groups laid side by side; within group h (tiles ntg, tokens th=128*ntg),
    flat column off_h + c*th + i*P + q holds x[tok0 + q*ntg + i, c*P + p]
    at partition row p.
    """
    plan, _, _ = _plans(ts)
    blocks = []
    tok0 = 0
    for ntg in plan:
        th = ntg * P
        a = xs[tok0:tok0 + th].reshape(P, ntg, DC, P)  # [q, i, c, p]
        a = a.transpose(3, 2, 1, 0)                    # [p, c, i, q]
        blocks.append(np.ascontiguousarray(a).reshape(P, DC * th))
        tok0 += th
    return np.concatenate(blocks, axis=1)


def prepare_in_maps(x, gate_weight, bias):
    x = np.asarray(x, dtype=np.float32)
    gw = np.asarray(gate_weight, dtype=np.float32)
    bb = np.ascontiguousarray(np.asarray(bias, dtype=np.float32)).reshape(1, E)

    ts = T // NCORES

    # W^T in device layout [P, c, e] = W[e, c*P + p]
    wt = np.ascontiguousarray(gw.T.reshape(DC, P, E).transpose(1, 0, 2))
    wh = wt.astype(BF16_NP).astype(np.float32)
    wl = (wt - wh).astype(BF16_NP)
    whl = np.concatenate([wh.astype(BF16_NP), wl], axis=2)  # [P, DC, 2E]
    if KMODE == "fp8":
        w2 = (wh / S_LO).astype(BF16_NP)  # exact exponent shift
    else:
        w2 = wh.astype(BF16_NP)
    # stream-ordered W arena: [whl c0-8 | w2 c0-8 | whl c8-32 | w2 c8-32]
    wt_all = np.concatenate([
        whl[:, :8].reshape(P, -1), w2[:, :8].reshape(P, -1),
        whl[:, 8:].reshape(P, -1), w2[:, 8:].reshape(P, -1),
    ], axis=1)
    wt_all = np.ascontiguousarray(wt_all)

    in_maps = []
    for cid in range(NCORES):
        xt = _tile_xt(x[cid * ts:(cid + 1) * ts], ts)
        xh = xt.astype(np.float16)
        xr = xt - xh.astype(np.float32)
        if KMODE == "fp8":
            xl = (xr * S_LO).astype(FP8_NP)
        else:
            xl = xr.astype(BF16_NP)
        in_maps.append({
            "xt_hi": xh,
            "xt_lo": xl,
            "wt_all": wt_all,
            "bias": bb,
        })
    return in_maps


def kernel(x, gate_weight, bias):
    ts = T // NCORES
    nc = _get_nc(ts)
    in_maps = prepare_in_maps(x, gate_weight, bias)
    res = run_bass_kernel_spmd(nc, in_maps, core_ids=list(range(NCORES)))
    buf = np.concatenate([r["out_pk"] for r in res.results], axis=0)
    idx = np.ascontiguousarray(buf[:, :K])
    wts = np.ascontiguousarray(buf[:, K:]).view(np.float32)
    return idx, wts


# revision 57
# speedup vs baseline: 1.0612x; 1.0612x over previous
"""MiniMax MoE gate (sigmoid + bias, top-8, normalized weights) on 8 TRN2 cores.

Full inputs in, full outputs out. Data-parallel over tokens: each core gets
1024 of the 8192 tokens; gate weight and bias are replicated.

Final strategy (DMA-roofline oriented; HW-measured 61.6us vs 86.9us
baseline, rel err 8.4e-3 vs the 2e-2 gate):
  - x ships as fp16 hi (2B) + scaled-fp8 lo (1B) = 3 B/elem instead of the
    4 B/elem bf16 hi/lo pair: xl8 = e4m3((x - fp16(x)) * 2^13). The third
    matmul term runs mixed-dtype (fp8 stationary x bf16 moving W*2^-13), so
    the product lands at the exact fp32 scale and accumulates into the same
    PSUM tile - no epilogue fixup. fp16 hi (10 mantissa bits vs bf16's 7)
    keeps the gate error at baseline level while cutting x bytes 25%
    (14.2MB/core total, ~40us at the ~358GB/s per-core HBM limit).
  - Per (chunk c, tile i) the PE runs TWO matmuls instead of three:
      mm1: xh[c,i] (stationary) x [Wh[c] | Wl[c]] (moving, N=128) -> psum[:,0:128]
      mm2: xl8[c,i] (stationary) x Wh[c]*2^-13 (moving bf16, N=64) -> psum[:,0:64]
    (one LDWEIGHTS covers both W terms). Epilogue folds the halves:
    logits = psum[:,0:64] + psum[:,64:128] (ACT copy stages one half to
    SBUF - DVE can read only one PSUM operand per op).
  - ALL input DMA rides the single SP HWDGE queue in exact PE-consumption
    order: W head, group-0 head, W tail, then per group interleaved hi/lo
    pieces. One queue sustains ~390GB/s; two queues race and starve the
    gating transfers, and SWDGE (gpsimd) hogs the SDMA engines. ACT
    carries only the outputs + epilogue ops; measured ACT_TABLE_LOAD and
    descriptor-gen (~0.6us per dma_start) otherwise delay the stream.
  - Per piece the PE runs all mm1s then all mm2s (the lo piece streams
    during the mm1 phase); the last piece of each group runs mm2s first
    so only its mm1s trail the final bytes. Each DMA piece gets its own
    SBUF tile. Small final groups shrink the post-DMA tail.
  - measured caveat: HWDGE dma_start triggers issue through 8 round-robin
    completion lanes, so only ~8 transfers can be in flight; piece sizes
    below ~0.5MB couple the stream to PE progress and slow it.
"""

import os

import numpy as np
import ml_dtypes

import concourse.bacc as bacc
import concourse.mybir as mybir
from concourse.bass_utils import run_bass_kernel_spmd
from concourse.tile import TileContext

T, D, E, K = 8192, 4096, 64, 8
NCORES = 8
P = 128
F32 = mybir.dt.float32
BF16 = mybir.dt.bfloat16
FP16 = mybir.dt.float16
FP8 = mybir.dt.float8e4
BF16_NP = ml_dtypes.bfloat16
FP8_NP = ml_dtypes.float8_e4m3fn
DC = D // P   # 32 contraction chunks
S_LO = 2.0 ** 13   # fp8 lo scale; w2 = Wh * 2^-13 exactly cancels it

# "fp8": x lo as scaled e4m3 (3 B/elem total); "bf16": x lo as bf16 (4 B/elem)
KMODE = os.environ.get("KMODE", "fp8")
# tiles (x128 tokens) per token-group, per core
GROUP_PLAN = tuple(int(v) for v in os.environ.get("KPLAN", "3,3,1,1").split(","))
# chunk-piece size of a group's x-hi stream: big groups get fine pieces
# (plenty of ring backlog), 1-tile groups coarser ones so the descriptor
# generator stays ahead of the drain
HI_SUB_BIG = int(os.environ.get("KSUBB", "8"))
HI_SUB_SMALL = int(os.environ.get("KSUBS", "16"))


def _plans(ts):
    """Per-group hi-piece chunk splits and lo-piece splits.

    Lean in the middle (fewest transfers - every transfer costs a trip
    through the 8-lane trigger window), fine at the head (PE start) and
    tail (post-DMA matmul latency).
    """
    nt = ts // P
    plan = list(GROUP_PLAN)
    if sum(plan) != nt:  # fallback for small test shards
        plan = [1] * nt
    subs, los = [], []
    for h, ntg in enumerate(plan):
        if h == 0:
            subs.append((8, 8, 16))  # fine head, merged back half
            los.append(((0, DC // 2), (DC // 2, DC)))
        elif h == len(plan) - 1:
            subs.append((16, 8, 8))  # fine tail pieces
            los.append(((0, DC),))
        else:
            subs.append(tuple(HI_SUB_SMALL for _ in range(DC // HI_SUB_SMALL)))
            los.append(((0, DC),))
    return plan, subs, los


def build_nc(ts):
    """Per-core program for a shard of `ts` tokens."""
    plan, subs, los = _plans(ts)
    nh = len(plan)
    ths = [p * P for p in plan]
    total_cols = DC * sum(ths)
    lo_dt, lo_np_bytes = (FP8, 1) if KMODE == "fp8" else (BF16, 2)

    nc = bacc.Bacc("TRN2", target_bir_lowering=False)
    # host-tiled layout (see prepare_in_maps): group blocks side by side;
    # within a group, row p holds all of partition p's data contiguously.
    xhd = nc.dram_tensor("xt_hi", [P, total_cols], FP16, kind="ExternalInput")
    xld = nc.dram_tensor("xt_lo", [P, total_cols], lo_dt, kind="ExternalInput")
    # W arena in stream order: [whl c0-8 | w2 c0-8 | whl c8-32 | w2 c8-32]
    # so the head and tail each load with ONE contiguous transfer
    WHH, W2H = 8 * 2 * E, 8 * E
    WHT, W2T = (DC - 8) * 2 * E, (DC - 8) * E
    wad = nc.dram_tensor(
        "wt_all", [P, WHH + W2H + WHT + W2T], BF16, kind="ExternalInput"
    )
    b = nc.dram_tensor("bias", [1, E], F32, kind="ExternalInput")
    # idx (8 x int32) and weights (8 x fp32-bitcast) pack into one row so
    # each group stores with a single DMA; the host splits them
    opd = nc.dram_tensor("out_pk", [ts, 2 * K], mybir.dt.int32, kind="ExternalOutput")

    with TileContext(nc) as tc:
        with (
            tc.tile_pool(name="const", bufs=1) as cpool,
            tc.tile_pool(name="xin", bufs=1) as xpool,
            tc.tile_pool(name="epi", bufs=4) as epool,
            tc.tile_pool(name="outb", bufs=3) as opool,
            tc.tile_pool(name="plogit", bufs=2, space="PSUM") as plogit,
        ):
            bias_row = cpool.tile([1, E], F32)
            nc.gpsimd.dma_start(out=bias_row, in_=b[:, :])
            bias_bc = cpool.tile([P, E], F32)
            nc.gpsimd.partition_broadcast(bias_bc, bias_row)

            # EVERYTHING streams on the single SP HWDGE queue, in exact PE
            # consumption order - one queue sustains the per-core HBM rate,
            # and its FIFO guarantees bytes land in the order the PE needs
            # them (two queues race and starve the gating transfers; SWDGE
            # hogs the SDMA engines). ACT carries no transfers at all: its
            # sequencer runs only the epilogue copies/sigmoids.
            wa = cpool.tile([P, WHH + W2H + WHT + W2T], BF16)
            whl_h = wa[:, 0:WHH].rearrange("p (c e) -> p c e", e=2 * E)
            w2_h = wa[:, WHH:WHH + W2H].rearrange("p (c e) -> p c e", e=E)
            whl_t = wa[:, WHH + W2H:WHH + W2H + WHT].rearrange(
                "p (c e) -> p c e", e=2 * E
            )
            w2_t = wa[:, WHH + W2H + WHT:].rearrange("p (c e) -> p c e", e=E)

            def WHL(c):
                return whl_h[:, c, :] if c < 8 else whl_t[:, c - 8, :]

            def W2V(c):
                return w2_h[:, c, :] if c < 8 else w2_t[:, c - 8, :]
            # One SBUF tile PER DMA PIECE: a piece's transfer then has no
            # relation to reads of other pieces, so the DMA stream never
            # waits on PE progress (tile-granular hazard tracking would
            # otherwise couple piece N+1's write to piece N's reads).
            xhps, xlps, hi_rs = [], [], []
            offs = []
            off = 0
            for h in range(nh):
                th = ths[h]
                rs = []
                c0 = 0
                for cs in subs[h]:
                    rs.append((c0, c0 + cs))
                    c0 += cs
                hi_rs.append(rs)
                xhps.append([
                    xpool.tile(
                        [P, b - a, th], FP16, tag=f"xh{h}_{pi}",
                        name=f"xh{h}_{pi}",
                    )
                    for pi, (a, b) in enumerate(rs)
                ])
                xlps.append([
                    xpool.tile(
                        [P, b - a, th], lo_dt, tag=f"xl{h}_{qi}",
                        name=f"xl{h}_{qi}",
                    )
                    for qi, (a, b) in enumerate(los[h])
                ])
                offs.append(off)
                off += DC * th

            def ld_x(t, dram, h, c0, c1, eng):   # x piece tile of group h
                th = ths[h]
                eng.dma_start(
                    out=t[:].rearrange("p c t -> p (c t)"),
                    in_=dram[:, offs[h] + c0 * th: offs[h] + c1 * th],
                )

            # W head + group-0 head first so matmuls start ASAP, then W
            # tail, then the groups in PE order. Within a group the hi and
            # lo pieces interleave in consumption order: the PE runs "all
            # mm1s of a piece, then its mm2s", so each lo piece streams
            # while the mm1s of its range execute. The final hi piece of a
            # group lands after the final lo piece (its mm1s run last).
            def x_pieces(h):
                hi = hi_rs[h]
                order = []
                li = 0
                lo = los[h]
                for pi, (a, b) in enumerate(hi):
                    if pi == len(hi) - 1 and li < len(lo):
                        order.extend(("lo", qi) for qi in range(li, len(lo)))
                        li = len(lo)
                    order.append(("hi", pi))
                    while li < len(lo) and lo[li][0] < b:
                        order.append(("lo", li))
                        li += 1
                return order

            # All input on the single SP ring in consumption order. Every
            # multi-ring variant measured slower (bulk hi/lo split 67.8us,
            # SWDGE lo-stream 70us, strict per-piece alternation 80.5us,
            # vs 62.8us single-ring): the second ring contends for the
            # shared SDMA engines and breaks the consumption-order FIFO
            # guarantee that keeps the gating transfers first.
            def emit_group(h, skip=0):
                for kind, pi in x_pieces(h)[skip:]:
                    if kind == "hi":
                        a, bb = hi_rs[h][pi]
                        ld_x(xhps[h][pi], xhd, h, a, bb, nc.sync)
                    else:
                        a, bb = los[h][pi]
                        ld_x(xlps[h][pi], xld, h, a, bb, nc.sync)
                    yield

            # (A SWDGE head-prefetch during the startup window measured
            # 64.6us vs 62.6us - the Q7 path does not actually beat the
            # ring and contends with it. Keep everything on the ring.)
            nc.sync.dma_start(
                out=wa[:, 0:WHH + W2H], in_=wad[:, 0:WHH + W2H]
            )
            for n, _ in enumerate(emit_group(0)):
                if n == 1:
                    # W tail right after the first hi+lo pieces
                    nc.sync.dma_start(
                        out=wa[:, WHH + W2H:], in_=wad[:, WHH + W2H:]
                    )
            for h in range(1, nh):
                for _ in emit_group(h):
                    pass

            tok0 = 0
            for h in range(nh):
                th, ntg = ths[h], plan[h]
                lgs = [
                    plogit.tile([P, 2 * E], F32, tag=f"lg{i}", name=f"lg_h{h}_{i}")
                    for i in range(ntg)
                ]
                # two-phase per piece, in DMA-arrival order: all mm1s of a
                # piece run while the matching lo piece streams in, then the
                # mm2s. Last piece reversed (mm2s first), so the only work
                # after a group's final hi bytes land is that piece's mm1s,
                # and each tile's final write is its stop-flagged mm1.
                for pi, (a, b) in enumerate(hi_rs[h]):
                    last_piece = pi == len(hi_rs[h]) - 1

                    def mm(ph, c, i, lg):
                        tsl = slice(i * P, (i + 1) * P)
                        if ph == "hi":
                            nc.tensor.matmul(
                                lg[:, :],
                                xhps[h][pi][:, c - a, tsl],
                                WHL(c),
                                start=(c == 0),
                                stop=(c == DC - 1),
                            )
                        else:
                            qi = next(
                                q for q, (la, lb) in enumerate(los[h])
                                if la <= c < lb
                            )
                            nc.tensor.matmul(
                                lg[:, 0:E],
                                xlps[h][qi][:, c - los[h][qi][0], tsl],
                                W2V(c),
                                start=False, stop=False,
                            )

                    if not last_piece:
                        for ph in ("hi", "lo"):
                            for c in range(a, b):
                                for i, lg in enumerate(lgs):
                                    mm(ph, c, i, lg)
                    else:
                        # lo phase first (its bytes landed earlier), then the
                        # hi phase TILE-major so tile 0's stop-matmul retires
                        # first and its epilogue releases ASAP
                        for c in range(a, b):
                            for i, lg in enumerate(lgs):
                                mm("lo", c, i, lg)
                        for i, lg in enumerate(lgs):
                            for c in range(a, b):
                                mm("hi", c, i, lg)

                obuf = opool.tile(
                    [P, ntg, 2 * K], mybir.dt.uint32, tag=f"ob{ntg}", name=f"ob{h}"
                )
                for i, lg in enumerate(lgs):
                    # only one PSUM operand allowed per DVE op: stage the Wl
                    # half through SBUF on the (idle) ACT engine
                    wlh = epool.tile([P, E], F32, tag="wlh")
                    nc.scalar.copy(out=wlh, in_=lg[:, E:2 * E])
                    ls = epool.tile([P, E], F32, tag="ls")
                    nc.vector.tensor_tensor(
                        out=ls, in0=lg[:, 0:E], in1=wlh,
                        op=mybir.AluOpType.add,
                    )
                    sc = epool.tile([P, E], F32, tag="sc")
                    nc.scalar.activation(
                        out=sc, in_=ls,
                        func=mybir.ActivationFunctionType.Sigmoid,
                    )
                    bi = epool.tile([P, E], F32, tag="bi")
                    nc.vector.tensor_tensor(
                        out=bi, in0=sc, in1=bias_bc, op=mybir.AluOpType.add
                    )
                    msel = epool.tile([P, K], F32, tag="msel")
                    nc.vector.max(out=msel, in_=bi)
                    nc.vector.max_index(
                        out=obuf[:, i, 0:K], in_max=msel, in_values=bi
                    )
                    # one-hot extraction of the selected raw scores, batched
                    # over all 8 slots with 0-stride broadcast APs. The
                    # compare runs in fp32 (tie-exactness feeds idx), but
                    # the mask/score product runs in bf16 for 2x DVE rate -
                    # it only perturbs the WEIGHTS (~2^-9 rel), which have
                    # 10x headroom vs the idx-dominated gate.
                    bi_b = bi[:].rearrange("p (o e) -> p o e", o=1).to_broadcast(
                        [P, K, E]
                    )
                    sc16 = epool.tile([P, E], BF16, tag="sc16")
                    nc.scalar.copy(out=sc16, in_=sc)
                    sc_b = sc16[:].rearrange(
                        "p (o e) -> p o e", o=1
                    ).to_broadcast([P, K, E])
                    msel_b = msel[:].rearrange(
                        "p (k o) -> p k o", o=1
                    ).to_broadcast([P, K, E])
                    oh8 = epool.tile([P, K, E], BF16, tag="oh8")
                    nc.vector.tensor_tensor(
                        out=oh8, in0=bi_b, in1=msel_b,
                        op=mybir.AluOpType.is_equal,
                    )
                    ohs = epool.tile([P, K, E], BF16, tag="ohs")
                    nc.vector.tensor_tensor(
                        out=ohs, in0=oh8, in1=sc_b, op=mybir.AluOpType.mult
                    )
                    ssel = epool.tile([P, K], F32, tag="ssel")
                    nc.vector.tensor_reduce(
                        out=ssel, in_=ohs,
                        axis=mybir.AxisListType.X, op=mybir.AluOpType.add,
                    )
                    ssum = epool.tile([P, 1], F32, tag="ssum")
                    nc.vector.tensor_reduce(
                        out=ssum, in_=ssel,
                        axis=mybir.AxisListType.X, op=mybir.AluOpType.add,
                    )
                    rsum = epool.tile([P, 1], F32, tag="rsum")
                    nc.vector.reciprocal(out=rsum, in_=ssum)
                    nc.vector.tensor_scalar_mul(
                        obuf[:, i, K:2 * K].bitcast(F32), ssel, rsum[:]
                    )

                # token at output partition q of tile i is tok0 + q*ntg + i.
                # Outputs ride the ACT ring (empty, so they land promptly)
                # and keep the SP ring pure-input.
                nc.scalar.dma_start(
                    out=opd[tok0:tok0 + th, :].rearrange(
                        "(q i) k -> q i k", i=ntg
                    ),
                    in_=obuf[:].bitcast(mybir.dt.int32),
                )
                tok0 += th

    nc.compile()
    return nc


_NC_CACHE = {}


def _get_nc(ts):
    if ts not in _NC_CACHE:
        _NC_CACHE[ts] = build_nc(ts)
    return _NC_CACHE[ts]


def _tile_xt(xs, ts):
    """[ts, D] fp32 -> [P, DC*ts] fp32 in the device layout.

    Groups laid side by side; within group h (tiles ntg, tokens th=128*ntg),
    flat column off_h + c*th + i*P + q holds x[tok0 + q*ntg + i, c*P + p]
    at partition row p.
    """
    plan, _, _ = _plans(ts)
    blocks = []
    tok0 = 0
    for ntg in plan:
        th = ntg * P
        a = xs[tok0:tok0 + th].reshape(P, ntg, DC, P)  # [q, i, c, p]
        a = a.transpose(3, 2, 1, 0)                    # [p, c, i, q]
        blocks.append(np.ascontiguousarray(a).reshape(P, DC * th))
        tok0 += th
    return np.concatenate(blocks, axis=1)


def prepare_in_maps(x, gate_weight, bias):
    x = np.asarray(x, dtype=np.float32)
    gw = np.asarray(gate_weight, dtype=np.float32)
    bb = np.ascontiguousarray(np.asarray(bias, dtype=np.float32)).reshape(1, E)

    ts = T // NCORES

    # W^T in device layout [P, c, e] = W[e, c*P + p]
    wt = np.ascontiguousarray(gw.T.reshape(DC, P, E).transpose(1, 0, 2))
    wh = wt.astype(BF16_NP).astype(np.float32)
    wl = (wt - wh).astype(BF16_NP)
    whl = np.concatenate([wh.astype(BF16_NP), wl], axis=2)  # [P, DC, 2E]
    if KMODE == "fp8":
        w2 = (wh / S_LO).astype(BF16_NP)  # exact exponent shift
    else:
        w2 = wh.astype(BF16_NP)
    # stream-ordered W arena: [whl c0-8 | w2 c0-8 | whl c8-32 | w2 c8-32]
    wt_all = np.concatenate([
        whl[:, :8].reshape(P, -1), w2[:, :8].reshape(P, -1),
        whl[:, 8:].reshape(P, -1), w2[:, 8:].reshape(P, -1),
    ], axis=1)
    wt_all = np.ascontiguousarray(wt_all)

    in_maps = []
    for cid in range(NCORES):
        xt = _tile_xt(x[cid * ts:(cid + 1) * ts], ts)
        xh = xt.astype(np.float16)
        xr = xt - xh.astype(np.float32)
        if KMODE == "fp8":
            xl = (xr * S_LO).astype(FP8_NP)
        else:
            xl = xr.astype(BF16_NP)
        in_maps.append({
            "xt_hi": xh,
            "xt_lo": xl,
            "wt_all": wt_all,
            "bias": bb,
        })
    return in_maps


def kernel(x, gate_weight, bias):
    ts = T // NCORES
    nc = _get_nc(ts)
    in_maps = prepare_in_maps(x, gate_weight, bias)
    res = run_bass_kernel_spmd(nc, in_maps, core_ids=list(range(NCORES)))
    buf = np.concatenate([r["out_pk"] for r in res.results], axis=0)
    idx = np.ascontiguousarray(buf[:, :K])
    wts = np.ascontiguousarray(buf[:, K:]).view(np.float32)
    return idx, wts
